# revision 1
# baseline (speedup 1.0000x reference)
"""AWD-LSTM Trainium2 kernel — 8-core SPMD, data-parallel over batch.

Problem: 3-layer LSTM (400->1152->1152->400), T=128, B=64, + decoder GEMM
to vocab 33278.  Full inputs in, full output out.

Sharding: batch 64 -> 8 per core.  Per core:
  A1: G0^T = [Wih0|b0] @ [x;1]^T      (batched GEMM over all T)
  B0: L0 recurrence over T=128 steps  (per-step W_hh matmuls, fp16)
  A2: G1^T from H0 history, B1, A3, B2 likewise
  C:  decoder GEMM (full vocab per core, batch-split rows) + bias
Everything fp16 on the PE (fp32 PSUM accumulate), cell state fp32.
Zero collectives; host pre-transposes/pads/casts inputs per core and
reassembles the full (128, 64, 33278) fp32 output.
"""
import sys, os, time
sys.path.insert(0, "/opt/trn_rl_repo")
import numpy as np

import concourse.bass as bass
import concourse.mybir as mybir
from concourse import bacc
from concourse.tile import TileContext

F16 = mybir.dt.float16
F32 = mybir.dt.float32
AF = mybir.ActivationFunctionType

# dims
T, B, EMB, HID, NTOK = 128, 64, 400, 1152, 33278
NCORE = 8
BS = B // NCORE            # 8 samples per core
EMBP = 512                 # padded emb (K for L0 input / decoder; per-gate pad for L2)
G01 = 4 * HID              # 4608 gates L0/L1
G2P = 4 * EMBP             # 2048 padded gates L2
VOCP = 33280               # padded vocab
MD = VOCP // 128           # 260 decoder m-tiles

# gate order in pytorch weights: i, f, g, o
GATE_SLICES = {}


def _mm_group_order(mtiles_per_gate):
    """m-tile issue order: i, g, f, o (so the elementwise chain starts early)."""
    i0, g0, f0, o0 = 0, 2 * mtiles_per_gate, mtiles_per_gate, 3 * mtiles_per_gate
    order = []
    for base, gate in ((i0, "i"), (g0, "g"), (f0, "f"), (o0, "o")):
        order.append((gate, list(range(base, base + mtiles_per_gate))))
    return order


def build_kernel(Tst=T, phases=7, n_unroll=None, repeat=1):
    """Tst: number of timesteps (parameterized for small-sim testing).
    phases: build only the first k of [A1,B0,A2,B1,A3,B2,C] (timing attribution).
    repeat: duplicate the whole pipeline (for slope timing)."""
    NT = Tst * BS                   # history columns (1024 for full problem)
    NCH = NT // 512 if NT >= 512 else 1   # n-chunks for batched GEMMs
    NCHW = min(512, NT)             # n-chunk width

    nc = bacc.Bacc("TRN2", target_bir_lowering=False)

    # ---- inputs (per-core, host-prepped layouts) ----
    xT = nc.dram_tensor("xT", [128, EMBP // 128, NT], F16, kind="ExternalInput")
    wih0 = nc.dram_tensor("wih0", [128, EMBP // 128, G01], F16, kind="ExternalInput")
    whh0 = nc.dram_tensor("whh0", [128, HID // 128, G01], F16, kind="ExternalInput")
    wih1 = nc.dram_tensor("wih1", [128, HID // 128, G01], F16, kind="ExternalInput")
    b1T = nc.dram_tensor("b1T", [128, G01 // 128], F32, kind="ExternalInput")
    whh1 = nc.dram_tensor("whh1", [128, HID // 128, G01], F16, kind="ExternalInput")
    wih2 = nc.dram_tensor("wih2", [128, HID // 128, G2P], F16, kind="ExternalInput")
    b2T = nc.dram_tensor("b2T", [128, G2P // 128], F32, kind="ExternalInput")
    whh2 = nc.dram_tensor("whh2", [128, EMBP // 128, G2P], F16, kind="ExternalInput")
    wdec = nc.dram_tensor("wdec", [128, EMBP // 128, VOCP], F16, kind="ExternalInput")
    bdec = nc.dram_tensor("bdec", [128, MD], F32, kind="ExternalInput")
    h00 = nc.dram_tensor("h00", [128, HID // 128 * BS], F16, kind="ExternalInput")
    h01 = nc.dram_tensor("h01", [128, HID // 128 * BS], F16, kind="ExternalInput")
    h02 = nc.dram_tensor("h02", [128, EMBP // 128 * BS], F16, kind="ExternalInput")
    c00 = nc.dram_tensor("c00", [128, HID // 128 * BS], F32, kind="ExternalInput")
    c01 = nc.dram_tensor("c01", [128, HID // 128 * BS], F32, kind="ExternalInput")
    c02 = nc.dram_tensor("c02", [128, EMBP // 128 * BS], F32, kind="ExternalInput")

    # ---- scratch DRAM ----
    g0T = nc.dram_tensor("g0T", [128, G01 // 128, NT], F32)
    g1T = nc.dram_tensor("g1T", [128, G01 // 128, NT], F32)
    g2T = nc.dram_tensor("g2T", [128, G2P // 128, NT], F32)
    outT = nc.dram_tensor("outT", [128, MD, NT], F16, kind="ExternalOutput")

    with TileContext(nc) as tc:
      with tc.tile_pool(name="hist", bufs=1) as histp:
        h0h = histp.tile([128, HID // 128, (Tst + 1) * BS], F16, name="h0h")
        h1h = histp.tile([128, HID // 128, (Tst + 1) * BS], F16, name="h1h")
        h2h = histp.tile([128, EMBP // 128, (Tst + 1) * BS], F16, name="h2h")
        nc.sync.dma_start(out=h0h[:, :, 0:BS], in_=h00[:])
        nc.sync.dma_start(out=h1h[:, :, 0:BS], in_=h01[:])
        nc.sync.dma_start(out=h2h[:, :, 0:BS], in_=h02[:])
        steps = [
            lambda: _input_gemm(nc, tc, "a1", wih0, EMBP // 128, None, xT, g0T, G01 // 128, NCH, NCHW, None),
            lambda: _recurrence(nc, tc, "b0", whh0, HID // 128, G01 // 128, g0T, c00, h0h, Tst),
            lambda: _input_gemm(nc, tc, "a2", wih1, HID // 128, h0h, None, g1T, G01 // 128, NCH, NCHW, b1T),
            lambda: _recurrence(nc, tc, "b1", whh1, HID // 128, G01 // 128, g1T, c01, h1h, Tst),
            lambda: _input_gemm(nc, tc, "a3", wih2, HID // 128, h1h, None, g2T, G2P // 128, NCH, NCHW, b2T),
            lambda: _recurrence(nc, tc, "b2", whh2, EMBP // 128, G2P // 128, g2T, c02, h2h, Tst),
            lambda: _decoder(nc, tc, wdec, bdec, h2h, outT, NCH, NCHW),
        ]
        for f in steps[:phases]:
            f()
        if phases < 7:
            with tc.tile_pool(name="tz", bufs=1) as tz:
                z = tz.tile([128, 8], F16, name="tz_z")
                nc.vector.memset(z[:], 0.0)
                nc.sync.dma_start(out=outT[:, 0, 0:8], in_=z[:])

    nc.finalize()
    return nc


def _input_gemm(nc, tc, pname, w_dram, ktiles, hist_sb, x_dram, gT, mtiles, NCH, NCHW, bias_dram):
    """gT[:, m, n] = sum_k w[:, k, m*128:].T @ rhs[:, k, n-chunk] (+ bias per row).
    rhs: xT DRAM input or SBUF h-history (cols BS.. are h_1..h_T)."""
    with (
        tc.tile_pool(name=f"{pname}_w", bufs=1) as wp,
        tc.tile_pool(name=f"{pname}_x", bufs=3) as xp,
        tc.tile_pool(name=f"{pname}_st", bufs=6) as sp,
        tc.tile_pool(name=f"{pname}_ps", bufs=4, space="PSUM") as pp,
    ):
        w_sb = wp.tile([128, ktiles, w_dram.shape[2]], F16, name=f"{pname}_wsb")
        nc.sync.dma_start(out=w_sb[:], in_=w_dram[:])
        b_sb = None
        if bias_dram is not None:
            b_sb = wp.tile([128, mtiles], F32, name=f"{pname}_bsb")
            nc.sync.dma_start(out=b_sb[:], in_=bias_dram[:])
        for n in range(NCH):
            if hist_sb is None:
                rhs_sb = xp.tile([128, ktiles, NCHW], F16, tag=f"{pname}_rhs", name=f"{pname}_rhs")
                nc.sync.dma_start(out=rhs_sb[:], in_=x_dram[:, :, bass.ts(n, NCHW)])
            for m in range(mtiles):
                ps = pp.tile([128, NCHW], F32, tag=f"{pname}_ps", name=f"{pname}_ps")
                for k in range(ktiles):
                    rhs = rhs_sb[:, k, :] if hist_sb is None else \
                        hist_sb[:, k, BS + n * NCHW: BS + (n + 1) * NCHW]
                    nc.tensor.matmul(ps[:], w_sb[:, k, bass.ts(m, 128)], rhs,
                                     start=(k == 0), stop=(k == ktiles - 1))
                st = sp.tile([128, NCHW], F32, tag=f"{pname}_st", name=f"{pname}_st")
                if b_sb is None:
                    nc.vector.tensor_copy(st[:], ps[:])
                else:
                    nc.vector.tensor_scalar_add(st[:], ps[:], b_sb[:, m:m + 1])
                nc.sync.dma_start(out=gT[:, m, bass.ts(n, NCHW)], in_=st[:])


def _recurrence(nc, tc, pname, whh_dram, khtiles, mtiles, gT, c0_dram, hist_sb, Tst):
    mg = mtiles // 4
    W = mg * BS
    GW = 4 * W
    with (
        tc.tile_pool(name=f"{pname}_w", bufs=1) as wp,
        tc.tile_pool(name=f"{pname}_state", bufs=1) as statep,
        tc.tile_pool(name=f"{pname}_g", bufs=16) as gp,
        tc.tile_pool(name=f"{pname}_e", bufs=5) as ep,
        tc.tile_pool(name=f"{pname}_ps", bufs=2, space="PSUM") as pp,
    ):
        w_sb = wp.tile([128, khtiles, mtiles * 128], F16, name=f"{pname}_wsb")
        nc.sync.dma_start(out=w_sb[:], in_=whh_dram[:])
        cping = [statep.tile([128, khtiles * BS], F32, name=f"{pname}_c{i}") for i in range(2)]
        nc.sync.dma_start(out=cping[0][:], in_=c0_dram[:])

        for s_ in range(Tst):
            cp, cn = cping[s_ % 2], cping[(s_ + 1) % 2]
            g_sb = gp.tile([128, GW], F32, tag=f"{pname}_gsl", name=f"{pname}_gsl")
            nc.sync.dma_start(out=g_sb[:], in_=gT[:, :, bass.ts(s_, BS)])
            psg = {}
            for gate, mlist in _mm_group_order(mg):
                ps = pp.tile([128, W], F32, tag=f"{pname}_p{gate}", name=f"{pname}_p{gate}")
                psg[gate] = ps
                for j, m in enumerate(mlist):
                    for k in range(khtiles):
                        nc.tensor.matmul(ps[:, bass.ts(j, BS)],
                                         w_sb[:, k, bass.ts(m, 128)],
                                         hist_sb[:, k, bass.ts(s_, BS)],
                                         start=(k == 0), stop=(k == khtiles - 1))
            goff = {"i": 0, "f": W, "g": 2 * W, "o": 3 * W}
            def _gatev(gate, func):
                pre = ep.tile([128, W], F32, tag=f"{pname}_pre{gate}", name=f"{pname}_pre{gate}")
                nc.vector.tensor_add(pre[:], psg[gate][:], g_sb[:, goff[gate]:goff[gate] + W])
                act = ep.tile([128, W], F32, tag=f"{pname}_act{gate}", name=f"{pname}_act{gate}")
                nc.scalar.activation(act[:], pre[:], func)
                return act
            sig_i = _gatev("i", AF.Sigmoid)
            tanh_g = _gatev("g", AF.Tanh)
            t1 = ep.tile([128, W], F32, tag=f"{pname}_t1", name=f"{pname}_t1")
            nc.vector.tensor_mul(t1[:], sig_i[:], tanh_g[:])
            sig_f = _gatev("f", AF.Sigmoid)
            t2 = ep.tile([128, W], F32, tag=f"{pname}_t2", name=f"{pname}_t2")
            nc.vector.tensor_mul(t2[:], sig_f[:], cp[:])
            nc.vector.tensor_add(cn[:], t1[:], t2[:])
            tc_ = ep.tile([128, W], F32, tag=f"{pname}_tc", name=f"{pname}_tc")
            nc.scalar.activation(tc_[:], cn[:], AF.Tanh)
            sig_o = _gatev("o", AF.Sigmoid)
            nc.vector.tensor_mul(hist_sb[:, :, bass.ts(s_ + 1, BS)], sig_o[:], tc_[:])


def _decoder(nc, tc, wdec, bdec, h2h, outT, NCH, NCHW):
    MCH = 10
    NMC = MD // MCH
    with (
        tc.tile_pool(name="c_b", bufs=1) as hp,
        tc.tile_pool(name="c_w", bufs=3) as wp,
        tc.tile_pool(name="c_o", bufs=6) as op,
        tc.tile_pool(name="c_ps", bufs=4, space="PSUM") as pp,
    ):
        bd_sb = hp.tile([128, MD], F32, name="c_bd")
        nc.sync.dma_start(out=bd_sb[:], in_=bdec[:])
        for mc in range(NMC):
            w_sb = wp.tile([128, EMBP // 128, MCH * 128], F16, tag="c_w", name="c_w")
            nc.sync.dma_start(out=w_sb[:], in_=wdec[:, :, bass.ts(mc, MCH * 128)])
            for m in range(MCH):
                mg = mc * MCH + m
                for n in range(NCH):
                    ps = pp.tile([128, NCHW], F32, tag="c_ps", name="c_ps")
                    for k in range(EMBP // 128):
                        nc.tensor.matmul(ps[:], w_sb[:, k, bass.ts(m, 128)],
                                         h2h[:, k, BS + n * NCHW: BS + (n + 1) * NCHW],
                                         start=(k == 0), stop=(k == EMBP // 128 - 1))
                    ot = op.tile([128, NCHW], F16, tag="c_o", name="c_o")
                    nc.vector.tensor_scalar_add(ot[:], ps[:], bd_sb[:, mg:mg + 1])
                    nc.sync.dma_start(out=outT[:, mg, bass.ts(n, NCHW)], in_=ot[:])


# ================= host side =================

def _prep_core_inputs(inputs, core, Tst=T):
    """Build the per-core input dict (numpy) from full-problem inputs."""
    s = slice(core * BS, (core + 1) * BS)
    f16, f32 = np.float16, np.float32

    def padK(a, K):       # pad rows (axis 0) to K
        out = np.zeros((K, a.shape[1]), np.float32)
        out[: a.shape[0]] = a
        return out

    # x^T with bias-ones row at EMB
    x = np.asarray(inputs["x"])[:Tst, s, :]                   # [Tst, BS, 400]
    xT = x.transpose(2, 0, 1).reshape(EMB, Tst * BS)          # [400, NT]
    xTp = np.zeros((EMBP, Tst * BS), np.float32)
    xTp[:EMB] = xT
    xTp[EMB] = 1.0
    d = {}
    d["xT"] = xTp.reshape(EMBP // 128, 128, Tst * BS).transpose(1, 0, 2).astype(f16)

    def ktile(a):         # [K, M] -> [128, K//128, M]
        K = a.shape[0]
        return a.reshape(K // 128, 128, a.shape[1]).transpose(1, 0, 2)

    # L0
    b0 = np.asarray(inputs["b_ih0"]) + np.asarray(inputs["b_hh0"])
    w = padK(np.asarray(inputs["W_ih0"]).T, EMBP)             # [512, 4608]
    w[EMB] = b0
    d["wih0"] = ktile(w).astype(f16)
    d["whh0"] = ktile(np.asarray(inputs["W_hh0"]).T.astype(np.float32)).astype(f16)
    # L1 (bias row at HID)
    b1 = np.asarray(inputs["b_ih1"]) + np.asarray(inputs["b_hh1"])
    d["wih1"] = ktile(np.asarray(inputs["W_ih1"]).T.astype(np.float32)).astype(f16)
    d["b1T"] = b1.reshape(G01 // 128, 128).T.astype(f32)
    d["whh1"] = ktile(np.asarray(inputs["W_hh1"]).T.astype(np.float32)).astype(f16)
    # L2: reorder gate rows 4x400 -> 4x512 padded
    def gate_pad_rows(a):                                     # [1600, X] -> [2048, X]
        out = np.zeros((G2P, a.shape[1]), np.float32)
        for g in range(4):
            out[g * EMBP: g * EMBP + EMB] = a[g * EMB: (g + 1) * EMB]
        return out
    b2 = np.asarray(inputs["b_ih2"]) + np.asarray(inputs["b_hh2"])
    w = gate_pad_rows(np.asarray(inputs["W_ih2"]).astype(np.float32)).T  # [1152, 2048]
    d["wih2"] = ktile(w).astype(f16)
    b2p = gate_pad_rows(b2[:, None].astype(np.float32))[:, 0]
    d["b2T"] = b2p.reshape(G2P // 128, 128).T.astype(f32)
    w = np.asarray(inputs["W_hh2"]).T                          # [400, 1600]
    w = gate_pad_rows(w.T).T                                   # [400, 2048]
    w = padK(w, EMBP)                                          # [512, 2048]
    d["whh2"] = ktile(w).astype(f16)
    # decoder
    w = padK(np.asarray(inputs["W_dec"]).T, EMBP)              # [512, 33278]
    wp = np.zeros((EMBP, VOCP), np.float32)
    wp[:, :NTOK] = w
    d["wdec"] = ktile(wp).astype(f16)
    bd = np.zeros((VOCP,), np.float32)
    bd[:NTOK] = np.asarray(inputs["b_dec"])
    d["bdec"] = bd.reshape(MD, 128).T.astype(f32)              # [128, MD]
    # states: [BS, D] -> [128, Dtiles*BS]  (tile k at cols k*BS)
    def stateT(a, D, DP):
        aT = np.zeros((DP, BS), np.float32)
        aT[:D] = np.asarray(a)[s].T
        return aT.reshape(DP // 128, 128, BS).transpose(1, 0, 2).reshape(128, -1)
    d["h00"] = stateT(inputs["h0_0"], HID, HID).astype(f16)
    d["h01"] = stateT(inputs["h0_1"], HID, HID).astype(f16)
    d["h02"] = stateT(inputs["h0_2"], EMB, EMBP).astype(f16)
    d["c00"] = stateT(inputs["c0_0"], HID, HID).astype(f32)
    d["c01"] = stateT(inputs["c0_1"], HID, HID).astype(f32)
    d["c02"] = stateT(inputs["c0_2"], EMB, EMBP).astype(f32)
    return d


class SpmdRunner:
    def __init__(self, nc, n_cores=8, donate=False):
        import jax
        from jax.sharding import Mesh, PartitionSpec
        from jax.experimental.shard_map import shard_map
        from concourse.bass2jax import (_bass_exec_p, install_neuronx_cc_hook,
                                        partition_id_tensor)
        self.jax = jax
        install_neuronx_cc_hook()
        self.n_cores = n_cores
        partition_name = nc.partition_id_tensor.name if nc.partition_id_tensor else None
        in_names, out_names, out_avals, zero_outs = [], [], [], []
        for alloc in nc.m.functions[0].allocations:
            if not isinstance(alloc, mybir.MemoryLocationSet):
                continue
            name = alloc.memorylocations[0].name
            if alloc.kind == "ExternalInput":
                if name != partition_name:
                    in_names.append(name)
            elif alloc.kind == "ExternalOutput":
                out_names.append(name)
                shape = tuple(alloc.tensor_shape)
                dtype = mybir.dt.np(alloc.dtype)
                out_avals.append(jax.core.ShapedArray(shape, dtype))
                zero_outs.append(np.zeros(shape, dtype))
        self.in_names, self.out_names = list(in_names), out_names
        self.out_avals, self.zero_outs = out_avals, zero_outs
        n_params = len(in_names)
        self.n_params = n_params
        all_in = in_names + out_names
        if partition_name is not None:
            all_in.append(partition_name)

        def _body(*args):
            operands = list(args)
            if partition_name is not None:
                operands.append(partition_id_tensor())
            outs = _bass_exec_p.bind(
                *operands, out_avals=tuple(out_avals), in_names=tuple(all_in),
                out_names=tuple(out_names), lowering_input_output_aliases=(),
                sim_require_finite=True, sim_require_nnan=True, nc=nc)
            return tuple(outs)

        devices = jax.devices()[:n_cores]
        self.mesh = Mesh(np.asarray(devices), ("core",))
        in_specs = (PartitionSpec("core"),) * (n_params + len(out_names))
        out_specs = (PartitionSpec("core"),) * len(out_names)
        kw = dict(keep_unused=True)
        if donate:
            kw["donate_argnums"] = tuple(range(n_params, n_params + len(out_names)))
        self.fn = jax.jit(shard_map(_body, mesh=self.mesh, in_specs=in_specs,
                                    out_specs=out_specs, check_rep=False), **kw)
        self.PartitionSpec = PartitionSpec

    def put_inputs(self, in_maps):
        jax = self.jax
        per_core = [[np.asarray(m[name]) for name in self.in_names] for m in in_maps]
        concat_in = [np.concatenate([per_core[c][i] for c in range(self.n_cores)], axis=0)
                     for i in range(self.n_params)]
        concat_zeros = [np.zeros((self.n_cores * z.shape[0], *z.shape[1:]), z.dtype)
                        for z in self.zero_outs]
        sharding = jax.sharding.NamedSharding(self.mesh, self.PartitionSpec("core"))
        self._dev_args = [jax.device_put(a, sharding) for a in concat_in + concat_zeros]
        jax.block_until_ready(self._dev_args)

    def run(self):
        out = self.fn(*self._dev_args)
        self.jax.block_until_ready(out)
        return out

    def fetch(self, out):
        res = []
        for c in range(self.n_cores):
            d = {}
            for i, name in enumerate(self.out_names):
                a = np.asarray(out[i]).reshape(self.n_cores, *self.out_avals[i].shape)
                d[name] = a[c]
            res.append(d)
        return res


_CACHE = {}

def _get_runner():
    if "r" not in _CACHE:
        nc = build_kernel()
        _CACHE["r"] = SpmdRunner(nc)
    return _CACHE["r"]


def assemble_output(per_core_outs, Tst=T):
    """per-core outT [128, MD, NT] -> full (T, B, NTOK) fp32."""
    full = np.empty((Tst, B, NTOK), np.float32)
    for c in range(len(per_core_outs)):
        a = per_core_outs[c]["outT"].astype(np.float32)    # [128, 260, NT]
        a = a.reshape(128, MD, Tst, BS).transpose(2, 3, 1, 0)  # [T, BS, MD, 128]
        a = a.reshape(Tst, BS, VOCP)[:, :, :NTOK]
        full[:, c * BS:(c + 1) * BS] = a
    return full


def kernel(**inputs) -> np.ndarray:
    r = _get_runner()
    in_maps = [_prep_core_inputs(inputs, c) for c in range(NCORE)]
    r.put_inputs(in_maps)
    out = r.run()
    per_core = r.fetch(out)
    return assemble_output(per_core)


if __name__ == "__main__":
    t0 = time.time()
    nc = build_kernel()
    print("build ok", time.time() - t0, flush=True)

